# revision 8
# baseline (speedup 1.0000x reference)
"""V3: 4-queue SWDGE dma_gather (NIDX=1024) + restructured min/max trees.

Same data layout as the proven baseline: shard over G (each of 8 cores
handles 256 g'; x.T replicated). Partition p holds g' in {2p, 2p+1}; grp
c = gl*32 + s (gl = g' parity). 64 gather calls: call (lb, l) gathers
literal l of grps 8lb..8lb+8 for all 128 partitions (1024 idx, list pos
j = i*128 + p), round-robin over SWDGE queues 0-3.

V3 changes vs baseline:
- num_idxs register hoisted once (drops 64 per-call MOVEs).
- idx DMA in 8 per-batch splits so call 0 only waits on split 0.
- max-over-s restructured into per-batch partial trees + a tiny cross-
  batch tree, shortening the critical chain after the last gather.
softand ~ min tree, softor ~ max tree (error ~ gamma*ln32, cancelling).
"""

import numpy as np

import concourse.bacc as bacc
import concourse.bass as bass
import concourse.tile as tile
from concourse import mybir
from concourse.bass_utils import run_bass_kernel_spmd

B, G, S, L = 64, 2048, 32, 8
NCORES = 8
GSH = G // NCORES  # 256 g' per core
NIDX = 1024  # indices per dma_gather call (descriptor-ring-safe max)
NQ = 4  # SWDGE queues
NBATCH = 8  # batches of 8 grps; NBATCH*L = 64 calls
GRP_PER_BATCH = 8
COLS_PER_CALL = NIDX // 16  # 64 idx columns
IDX_COLS = NBATCH * L * COLS_PER_CALL  # 4096

_nc_cache = None
last_result = None


def _v(t, dims, off=0):
    return bass.AP(tensor=t.tensor, offset=t.offset + off, ap=[list(t.ap[0])] + dims)


def _build_nc():
    f32 = mybir.dt.float32
    nc = bacc.Bacc("TRN2", target_bir_lowering=False, num_swdge_queues=NQ)
    tbl_in = nc.dram_tensor("tbl", [G, B], f32, kind="ExternalInput")  # x.T
    idx_in = nc.dram_tensor("idx", [128, IDX_COLS], mybir.dt.int16, kind="ExternalInput")
    c_out = nc.dram_tensor("c", [128, 128], f32, kind="ExternalOutput")

    with tile.TileContext(nc) as tc:
        with (
            tc.tile_pool(name="singles", bufs=1) as singles,
            tc.tile_pool(name="gath", bufs=6) as gath,
            tc.tile_pool(name="work", bufs=3) as work,
        ):
            idxs = singles.tile([128, IDX_COLS], mybir.dt.int16)
            spl = IDX_COLS // NBATCH  # 512 cols per batch
            for lb in range(NBATCH):
                nc.sync.dma_start(
                    out=idxs[:, lb * spl : (lb + 1) * spl],
                    in_=idx_in[:, lb * spl : (lb + 1) * spl],
                )
            part = singles.tile([128, NBATCH, B], f32)  # per-batch max over s
            nidreg = nc.gpsimd.to_reg(NIDX)
            # warm-up: tiny gather hides the first-call cold cost (~6.5us)
            # under the idx DMA transfers
            idxz = singles.tile([128, 1], mybir.dt.int16)
            nc.vector.memset(idxz, 0)
            scrap = singles.tile([128, 1, B], f32)
            nc.gpsimd.dma_gather(
                scrap, tbl_in[:, :], idxz, num_idxs=16, num_idxs_reg=16,
                elem_size=B, queue_num=3,
            )
            for lb in range(NBATCH):
                gt = gath.tile([128, L, GRP_PER_BATCH, B], f32, tag="gt")
                for l in range(L):
                    c = lb * L + l
                    nc.gpsimd.dma_gather(
                        gt[:, l, :, :],
                        tbl_in[:, :],
                        idxs[:, c * COLS_PER_CALL : (c + 1) * COLS_PER_CALL],
                        num_idxs=NIDX,
                        num_idxs_reg=nidreg,
                        elem_size=B,
                        queue_num=c % NQ,
                    )
                # elementwise min tree over the 8 per-l tiles (pairwise, so
                # each op can fire as soon as its two transfers land)
                m0 = work.tile([128, GRP_PER_BATCH, B], f32, tag="m0")
                nc.vector.tensor_tensor(
                    out=m0, in0=gt[:, 0], in1=gt[:, 1], op=mybir.AluOpType.min
                )
                m1 = work.tile([128, GRP_PER_BATCH, B], f32, tag="m1")
                nc.vector.tensor_tensor(
                    out=m1, in0=gt[:, 2], in1=gt[:, 3], op=mybir.AluOpType.min
                )
                m2 = work.tile([128, GRP_PER_BATCH, B], f32, tag="m2")
                nc.vector.tensor_tensor(
                    out=m2, in0=gt[:, 4], in1=gt[:, 5], op=mybir.AluOpType.min
                )
                m3 = work.tile([128, GRP_PER_BATCH, B], f32, tag="m3")
                nc.vector.tensor_tensor(
                    out=m3, in0=gt[:, 6], in1=gt[:, 7], op=mybir.AluOpType.min
                )
                m4 = work.tile([128, GRP_PER_BATCH, B], f32, tag="m4")
                nc.vector.tensor_tensor(out=m4, in0=m0, in1=m1, op=mybir.AluOpType.min)
                m5 = work.tile([128, GRP_PER_BATCH, B], f32, tag="m5")
                nc.vector.tensor_tensor(out=m5, in0=m2, in1=m3, op=mybir.AluOpType.min)
                vvp = work.tile([128, GRP_PER_BATCH, B], f32, tag="vvp")
                nc.vector.tensor_tensor(out=vvp, in0=m4, in1=m5, op=mybir.AluOpType.min)
                # partial max over this batch's 8 s-columns -> part[:, lb]
                t4 = work.tile([128, 4, B], f32, tag="t4")
                nc.vector.tensor_tensor(
                    out=t4, in0=vvp[:, 0:4], in1=vvp[:, 4:8], op=mybir.AluOpType.max
                )
                t2 = work.tile([128, 2, B], f32, tag="t2")
                nc.vector.tensor_tensor(
                    out=t2, in0=t4[:, 0:2], in1=t4[:, 2:4], op=mybir.AluOpType.max
                )
                if lb % 4 == 0:  # first batch of half: start running max
                    nc.vector.tensor_tensor(
                        out=part[:, lb], in0=t2[:, 0], in1=t2[:, 1],
                        op=mybir.AluOpType.max,
                    )
                else:  # fold this batch into the running max
                    pl = work.tile([128, B], f32, tag="pl")
                    nc.vector.tensor_tensor(
                        out=pl, in0=t2[:, 0], in1=t2[:, 1], op=mybir.AluOpType.max
                    )
                    nc.vector.tensor_tensor(
                        out=part[:, lb], in0=pl, in1=part[:, lb - 1],
                        op=mybir.AluOpType.max,
                    )
                if lb % 4 == 3:  # running max complete for half gl = lb // 4
                    gl = lb // 4
                    nc.sync.dma_start(
                        out=c_out[:, gl * B : (gl + 1) * B], in_=part[:, lb]
                    )
    nc.finalize()
    return nc


def _prep_inputs(x: np.ndarray, I_i: np.ndarray):
    """Host-side layout: x transposed; per-core wrapped idx tensors."""
    tbl = np.ascontiguousarray(x.astype(np.float32, copy=False).T)  # [G, B]
    idx_maps = []
    I = np.asarray(I_i)
    for k in range(NCORES):
        Ik = I[k * GSH : (k + 1) * GSH]  # [256, 32, 8] values in [0, G)
        Ikr = Ik.reshape(128, 2, S, L)  # [p, gl, s, l]
        # grp c2 = gl*32 + s; call (lb, l) covers grps 8lb..8lb+8
        grp = np.transpose(Ikr, (1, 2, 3, 0)).reshape(2 * S, L, 128)  # [c2, l, p]
        idx_w = np.empty((16, IDX_COLS), dtype=np.int16)
        for c in range(NBATCH * L):
            lb, l = c // L, c % L
            flat = grp[
                lb * GRP_PER_BATCH : (lb + 1) * GRP_PER_BATCH, l, :
            ].reshape(NIDX)  # j = i2*128+p
            W = flat.reshape(COLS_PER_CALL, 16).T.astype(np.int16)  # [r, col]
            idx_w[:, c * COLS_PER_CALL : (c + 1) * COLS_PER_CALL] = W
        idx_maps.append(np.tile(idx_w, (8, 1)))
    return tbl, idx_maps


def kernel(x: np.ndarray, I_i: np.ndarray) -> np.ndarray:
    global _nc_cache, last_result
    if _nc_cache is None:
        _nc_cache = _build_nc()
    nc = _nc_cache
    tbl, idx_maps = _prep_inputs(x, I_i)
    in_maps = [{"tbl": tbl, "idx": idx_maps[k]} for k in range(NCORES)]
    res = run_bass_kernel_spmd(nc, in_maps, core_ids=list(range(NCORES)))
    last_result = res
    C = np.empty((B, G), dtype=np.float32)
    for k in range(NCORES):
        C[:, k * GSH : (k + 1) * GSH] = _assemble_core0(res.results[k]["c"])
    return C


def _assemble_core0(o: np.ndarray) -> np.ndarray:
    """Device 'c' tensor [128, 128] -> C[:, :GSH] slice for one core."""
    o = o.reshape(128, 2, B)  # [p, gl, b]
    return np.transpose(o, (2, 0, 1)).reshape(B, GSH)
